# revision 3
# baseline (speedup 1.0000x reference)
"""Trainium2 Bass kernel v2 for nn_DisentangleEncoder (B=64, L=200, D=256, K=8).

Data-parallel over batch: 8 sequences per NeuronCore x 8 cores.

Key math (per branch, per sequence b, x = item_emb[b]):
  mu1/var1  = stats of x over D
  lga       = (x - mu1) * g5                  [NO rstd1 -- folded elsewhere]
  score     = softmax_K( exp(SCALE*rstd1*(lgaT @ m2)) * c2f )
              with m2[d,k] = ln2[k,d]*g1[d]/g5[d], c2f[k] = exp(SCALE*(b1.ln2[k]))
  pa        = x + pos;  xpfn = (pa - mu4) * rstd4
  khT       = transpose(xpfn)*g4 + b4         [D on partitions]
  zr        = relu(W_w @ khT + W_b)
  wlogit[l] = q . khT[:,l] + q . zr[:,l]      (keyv add folded into PE accum)
  q         = LN(x[b,sl-1] + pos[sl-1] + rou)*g3 + b3   (gathers done on host)
  w         = softmax_L(wlogit * SCALE)
  cc[k,l]   = score[l,k] * w[l]
  alpha     = cc / sqrt(cc^2*var1 + eps)      [exact: LN(c*x) = alpha*lga + b5]
Final: out[k,l,:] = alphaL[k,l]*lgaL[l,:] + alphaG[k,l]*lgaG[l,:] + 2*b5
  computed on the PE as diag(alpha) matmuls in bf16 accumulated in PSUM;
  the 2*b5 term rides along as a 101st contraction row (ones in the diag
  stationary, b5 replicated in partition 100 of the lng tile).

All sqrt/rsqrt via exp(+-0.5*ln(v)) so ACT needs one table set.
"""

import numpy as np
from contextlib import ExitStack

import concourse.bacc as bacc
import concourse.bass as bass
import concourse.tile as tile
from concourse import mybir
from concourse.bass_utils import run_bass_kernel_spmd

# Route all Exp/Ln activations to the single `natural_log_exp_and_others`
# table set (one ACT_TABLE_LOAD instead of ~2.7us per switch).
_orig_gat = bacc.get_activation_tables


def _gat_joint_exp_ln(arch):
    tabs = _orig_gat(arch)
    out = {}
    for name, fns in tabs.items():
        fns = set(fns)
        if name != "natural_log_exp_and_others":
            fns.discard(mybir.ActivationFunctionType.Exp)
            fns.discard(mybir.ActivationFunctionType.Ln)
        out[name] = fns
    return out


bacc.get_activation_tables = _gat_joint_exp_ln

B, L, D, K = 64, 200, 256, 8
NCORES = 8
BPC = B // NCORES
EPS = 1e-5
SCALE = 1.0 / float(np.sqrt(D))
F32 = mybir.dt.float32
F32R = mybir.dt.float32r
BF16 = mybir.dt.bfloat16
LC = 100                  # l-chunk size; L = 2*LC
NS = 2 * BPC              # streams per core (seq x branch)
AX = mybir.AxisListType.X
OP = mybir.AluOpType
AF = mybir.ActivationFunctionType


def _bc(ap, p):
    """Broadcast a DRAM AP across p partitions (partition-step 0)."""
    return bass.AP(tensor=ap.tensor, offset=ap.offset, ap=[[0, p]] + list(ap.ap))


def _b0(ap, n=K):
    """Replace the (size-1) last axis of an AP with a step-0 broadcast of n."""
    a = list(ap.ap)
    a[-1] = [0, n]
    return bass.AP(tensor=ap.tensor, offset=ap.offset, ap=a)


def _b0a(ap, n=K):
    """Append a step-0 broadcast axis of size n to an AP."""
    a = list(ap.ap) + [[0, n]]
    return bass.AP(tensor=ap.tensor, offset=ap.offset, ap=a)


def _bins(ap, pos, n):
    """Insert a step-0 broadcast axis of size n at free-dim position pos."""
    a = list(ap.ap)
    a.insert(pos, [0, n])
    return bass.AP(tensor=ap.tensor, offset=ap.offset, ap=a)




# (name, free-offset, free-shape, partitions) inside the packed const blobs
F32_LAYOUT = [
    ("pos",    0,           (2, D), LC),    # [LC, 2, D]
    ("g5bc",   2 * D,       (D,),   LC),
    ("c2fb",   3 * D,       (K,),   LC),
    ("g3bc",   3 * D + K,   (D,),   BPC),
    ("b3bc",   4 * D + K,   (D,),   BPC),
    ("g4",     5 * D + K,   (2,),   128),
    ("b4",     5 * D + K + 2, (2,), 128),
    ("wb",     5 * D + K + 4, (2,), 128),
    ("eyef",   5 * D + K + 6, (LC,), LC),
]
CF32 = 5 * D + K + 6 + LC
B16_LAYOUT = [
    ("m2tb",  0,            (2, K), 128),
    ("eyeb",  2 * K,        (LC,),  LC),
    ("eyex1", 2 * K + LC,   (LC,),  LC + 1),
    ("eyex2", 2 * K + 2 * LC, (LC,), LC + 1),
]
CB16 = 2 * K + 3 * LC


def _pack_consts(shared_np):
    """Build the two const blobs from a dict of named numpy arrays.
    Each entry is [p, *shape]; rows beyond p are left zero."""
    import ml_dtypes
    bf32 = np.zeros((128, CF32), np.float32)
    bb16 = np.zeros((128, CB16), ml_dtypes.bfloat16)
    for nm, off, shape, p in F32_LAYOUT:
        v = shared_np[nm].reshape(p, int(np.prod(shape)))
        bf32[:p, off : off + v.shape[1]] = v
    for nm, off, shape, p in B16_LAYOUT:
        v = shared_np[nm].reshape(p, int(np.prod(shape)))
        bb16[:p, off : off + v.shape[1]] = v
    return bf32, bb16

def _dram_x_ap(t, b):
    """DRAM AP for x[b] viewed as [LC part, 2 lt, D]."""
    return bass.AP(tensor=t, offset=b * L * D, ap=[[D, LC], [LC * D, 2], [1, D]])


def _dram_out_ap(t, b, lt):
    """DRAM AP for out[b, :, lt*LC:(lt+1)*LC, :] viewed as [LC part, K, D]."""
    return bass.AP(tensor=t, offset=b * K * L * D + lt * LC * D,
                   ap=[[D, LC], [L * D, K], [1, D]])


def _emit_consts(nc, tc, ctx, t):
    """All f32 consts live in one packed [128, CF32] blob (one DMA), all bf16
    consts in one [128, CB16] blob, wwt (f32r) in its own tensor. Entries in
    the returned dict are AP slices of the blobs."""
    cp = ctx.enter_context(tc.tile_pool(name="consts", bufs=1))
    c = {}
    bf32 = cp.tile([128, CF32], F32, name="c_bf32")
    nc.sync.dma_start(out=bf32, in_=t["bf32"][:, :])
    bb16 = cp.tile([128, CB16], BF16, name="c_bb16")
    nc.sync.dma_start(out=bb16, in_=t["bb16"][:, :])
    wwt = cp.tile([128, 2, 2, 128], F32R, name="c_wwt")
    nc.sync.dma_start(out=wwt, in_=bass.AP(tensor=t["wwt"], offset=0,
                                           ap=[[D, 128], [128 * D, 2], [128, 2], [1, 128]]))
    c["wwt"] = wwt

    def _sl(blob, off, shape, p):
        stride = 1; rev = []
        for s in reversed(shape):
            rev.append([stride, s]); stride *= s
        dims = list(reversed(rev))
        return bass.AP(tensor=blob.tensor, offset=blob.offset + off,
                       ap=[[blob.ap[0][0], p]] + dims)

    def f32sl(off, shape, p=128):
        return _sl(bf32, off, shape, p)

    def b16sl(off, shape, p=128):
        return _sl(bb16, off, shape, p)

    for nm, off, shape, p in F32_LAYOUT:
        c[nm] = f32sl(off, shape, p)
    for nm, off, shape, p in B16_LAYOUT:
        c[nm] = b16sl(off, shape, p)
    qraw = cp.tile([BPC, 2, D], F32, name="c_qraw")
    nc.sync.dma_start(out=qraw, in_=t["qraw"][:, :, :])
    c["qraw"] = qraw
    posrou = cp.tile([BPC, D], F32, name="c_posrou")
    nc.sync.dma_start(out=posrou, in_=t["posrou"][:, :])
    c["posrou"] = posrou
    epsc = cp.tile([128, 1], F32, name="c_epsc")
    nc.gpsimd.memset(epsc, EPS)
    c["epsc"] = epsc
    lnsc = cp.tile([128, 1], F32, name="c_lnsc")
    nc.gpsimd.memset(lnsc, float(np.log(SCALE)))
    c["lnsc"] = lnsc
    onesc = cp.tile([LC, 1], F32, name="c_onesc")
    nc.gpsimd.memset(onesc, 1.0)
    c["onesc"] = onesc
    rc256 = cp.tile([LC, 1], F32, name="c_rc256")
    nc.gpsimd.memset(rc256, 1.0 / D)
    c["rc256"] = rc256
    onesr = cp.tile([1, LC], F32, name="c_onesr")
    nc.gpsimd.memset(onesr, 1.0)
    c["onesr"] = onesr
    return c


def _emit_body(nc, tc, c, pools, t, out_t, upto=12, n_hyb=2, diag_eng="gp",
               sgrp=2, dma_out=True, diag_mix=None, copy_mix=None, pair_t=True):
    """Emit the whole per-core computation.

    n_hyb:    number of k's (taken from the top) routed through the hybrid
              path (PE does the G-branch matmul, DVE stt fuses the L-branch
              multiply-add + PSUM read + SBUF write).
    diag_eng: 'gp' | 'dve' — engine that builds diag(alpha) stationaries.
    sgrp:     seqs per pipeline group (BPC % sgrp == 0).
    """
    blk, tmp, pst, psw, psA, op = pools

    def _pA(name):
        return psA.tile([128, 2, D], F32, name=name, tag="pA")
    xsrc = {0: t["xL"], 1: t["xG"]}
    # rotation lists: engines picked round-robin for diag builds / p10 copies
    if diag_mix is None:
        diag_mix = {"gp": (nc.gpsimd,), "dve": (nc.vector,)}[diag_eng]
    else:
        diag_mix = tuple({"gp": nc.gpsimd, "dve": nc.vector, "act": "act"}[e]
                         for e in diag_mix)
    if copy_mix is None:
        copy_mix = ("act", "act", "dve")
    _dcnt = [0]
    def _diag(dg, eyex, col):
        e = diag_mix[_dcnt[0] % len(diag_mix)]; _dcnt[0] += 1
        if e == "act":
            nc.scalar.activation(out=dg, in_=eyex, func=AF.Identity, bias=0.0,
                                 scale=col)
        elif e is nc.gpsimd:
            nc.gpsimd.tensor_tensor(out=dg, in0=eyex, in1=_b0(col, LC),
                                    op=OP.mult)
        else:
            e.tensor_scalar_mul(dg, eyex, col)
    _ccnt = [0]
    def _copy10(dst, srcp):
        e = copy_mix[_ccnt[0] % len(copy_mix)]; _ccnt[0] += 1
        if e == "act":
            nc.scalar.activation(out=dst, in_=srcp, func=AF.Identity, bias=0.0,
                                 scale=1.0)
        else:
            nc.vector.tensor_copy(dst, srcp)

    # ---- persistent tiles ----
    xa = blk.tile([128, 2, NS, D], F32, name="xa", tag="xa")
    pa = blk.tile([128, 2, NS, D], F32, name="pa", tag="pa")
    lng = blk.tile([128, 2, NS, D], BF16, name="lng", tag="lng")
    lgT = blk.tile([128, NS, 2, L], BF16, name="lgT", tag="lgT")
    khT = blk.tile([128, NS, 2, L], F32R, name="khT", tag="khT")
    mv1 = blk.tile([128, 2, NS, 2], F32, name="mv1", tag="mv1")
    mv4 = blk.tile([128, 2, NS, 2], F32, name="mv4", tag="mv4")
    rs1s = blk.tile([128, 2, NS, 1], F32, name="rs1s", tag="rs1s")
    rs4 = blk.tile([128, 2, NS, 1], F32, name="rs4", tag="rs4")
    sex = blk.tile([128, 2, NS, K], F32, name="sex", tag="sex")
    cca = blk.tile([128, 2, NS, K], F32, name="cca", tag="cca")
    alf = blk.tile([128, 2, NS, K], F32, name="alf", tag="alf")
    wex = blk.tile([LC, 2, NS], F32, name="wex", tag="wex")
    wn = blk.tile([LC, 2, NS], F32, name="wn", tag="wn")
    qp = blk.tile([BPC, 2, D], F32, name="qp", tag="qp")
    qT = blk.tile([128, 2, 2, BPC], F32, name="qT", tag="qT")

    # partition LC of lng = b5 row (per (lt,st) free offset); alpha row LC = 1.0
    nc.sync.dma_start(out=lng[96 : LC + 1, :, :, :], in_=t["b5rep"][:, :, :, :])
    nc.gpsimd.memset(alf[96:128, :, :, :], 1.0)

    # ---- q chain (global; tiny) ----
    nc.vector.tensor_tensor(
        out=qp, in0=c["qraw"], in1=_bins(c["posrou"], 1, 2), op=OP.add)
    q6 = tmp.tile([BPC, 2, 6], F32, name="q6", tag="q6", bufs=1)
    qmv = tmp.tile([BPC, 2, 2], F32, name="qmv", tag="qmv", bufs=1)
    for br in range(2):
        nc.vector.bn_stats(out=q6[:, br, :], in_=qp[:, br, :])
        nc.vector.bn_aggr(out=qmv[:, br, :], in_=q6[:, br, :])
    qle = tmp.tile([BPC, 2, 1], F32, name="qle", tag="qle", bufs=1)
    nc.scalar.activation(out=qle, in_=qmv[:, :, 1:2], func=AF.Ln,
                         bias=c["epsc"][:BPC, :], scale=1.0)
    qrs = tmp.tile([BPC, 2, 1], F32, name="qrs", tag="qrs", bufs=1)
    nc.scalar.activation(out=qrs, in_=qle, func=AF.Exp, bias=0.0, scale=-0.5)
    for br in range(2):
        nc.vector.tensor_scalar(qp[:, br, :], qp[:, br, :], qmv[:, br, 0:1],
                                qrs[:, br, 0:1], OP.subtract, OP.mult)
        nc.vector.tensor_mul(qp[:, br, :], qp[:, br, :], c["g3bc"])
        nc.vector.tensor_add(qp[:, br, :], qp[:, br, :], c["b3bc"])
        for dh in range(2):
            pqt = _pA(f"pq{br}{dh}")
            pq = pqt[:, 0, :BPC]
            nc.tensor.transpose(pq, qp[:, br, dh * 128 : (dh + 1) * 128],
                                c["eyef"][:BPC, :BPC])
            nc.scalar.activation(out=qT[:, dh, br, :], in_=pq, func=AF.Identity,
                                 bias=0.0, scale=1.0)

    # ---- all x loads up front (SP starts immediately; consts follow) ----
    for st in range(NS):
        b, br = st // 2, st % 2
        nc.sync.dma_start(out=xa[:LC, :, st, :], in_=_dram_x_ap(xsrc[br], b))

    ngrp = BPC // sgrp
    for g in range(ngrp):
        seqs = list(range(g * sgrp, (g + 1) * sgrp))
        sts = [2 * b + br for b in seqs for br in range(2)]
        gs0, gs1 = 2 * sgrp * g, 2 * sgrp * (g + 1)
        gssl = slice(gs0, gs1)               # contiguous group stream slice

        # ---- x stats (DVE) + pa with accumulated moments (GP) ----
        for st in sts:
            for lt in range(2):
                s6 = tmp.tile([LC, 6], F32, name=f"s6_{g}_{st}{lt}", tag="s6", bufs=4)
                nc.vector.bn_stats(out=s6, in_=xa[:LC, lt, st, :])
                nc.vector.bn_aggr(out=mv1[:LC, lt, st, :], in_=s6)
        for st in sts:
            for lt in range(2):
                nc.gpsimd.tensor_add(pa[:LC, lt, st, :], xa[:LC, lt, st, :],
                                     c["pos"][:, lt, :])
                s6 = tmp.tile([LC, 6], F32, name=f"s64_{g}_{st}{lt}", tag="s6",
                              bufs=4)
                nc.vector.bn_stats(out=s6, in_=pa[:LC, lt, st, :])
                nc.vector.bn_aggr(out=mv4[:LC, lt, st, :], in_=s6)
        # ---- rstd chains (batched per group) ----
        ns_g = 2 * sgrp
        for mv, rs, bias in ((mv1, rs1s, c["lnsc"]), (mv4, rs4, 0.0)):
            le = tmp.tile([LC, 2, ns_g, 1], F32, name=f"le_{g}", tag="le", bufs=2)
            nc.scalar.activation(out=le, in_=mv[:LC, :, gssl, 1:2], func=AF.Ln,
                                 bias=c["epsc"][:LC, :], scale=1.0)
            if isinstance(bias, float):
                nc.scalar.activation(out=rs[:LC, :, gssl, :], in_=le,
                                     func=AF.Exp, bias=bias, scale=-0.5)
            else:
                nc.scalar.activation(out=rs[:LC, :, gssl, :], in_=le,
                                     func=AF.Exp, bias=bias[:LC, :], scale=-0.5)
        # ---- lga (bf16) + xpfn in place ----
        for st in sts:
            for lt in range(2):
                nc.vector.scalar_tensor_tensor(
                    out=lng[:LC, lt, st, :], in0=xa[:LC, lt, st, :],
                    scalar=mv1[:LC, lt, st, 0:1], in1=c["g5bc"],
                    op0=OP.subtract, op1=OP.mult)
                nc.vector.tensor_scalar(
                    pa[:LC, lt, st, :], pa[:LC, lt, st, :],
                    mv4[:LC, lt, st, 0:1], rs4[:LC, lt, st, 0:1],
                    OP.subtract, OP.mult)
        if upto <= 4:
            continue

        # ---- transposes: lng -> lgT (bf16), xpfn -> khT (f32, g4/b4) ----
        if pair_t:
            for b in seqs:
                st0 = 2 * b
                for dh in range(2):
                    pb = pst.tile([128, 2, L], BF16, name=f"pb_{g}_{b}{dh}", tag="pTb")
                    for si in range(2):
                        for lt in range(2):
                            nc.tensor.transpose(
                                pb[:, si, lt * LC : (lt + 1) * LC],
                                lng[:LC, lt, st0 + si, dh * 128 : (dh + 1) * 128],
                                c["eyeb"])
                    nc.scalar.activation(out=lgT[:, st0 : st0 + 2, dh, :], in_=pb,
                                         func=AF.Identity, bias=0.0, scale=1.0)
                    pf = pst.tile([128, 2, L], F32, name=f"pf_{g}_{b}{dh}", tag="pTf")
                    for si in range(2):
                        for lt in range(2):
                            nc.tensor.transpose(
                                pf[:, si, lt * LC : (lt + 1) * LC],
                                pa[:LC, lt, st0 + si, dh * 128 : (dh + 1) * 128],
                                c["eyef"])
                    nc.scalar.activation(out=khT[:, st0 : st0 + 2, dh, :], in_=pf,
                                         func=AF.Identity, bias=c["b4"][:, dh : dh + 1],
                                         scale=c["g4"][:, dh : dh + 1])
        else:
            for st in sts:
                for dh in range(2):
                    pb = pst.tile([128, 2, L], BF16, name=f"pb1_{g}_{st}{dh}", tag="pTb")
                    for lt in range(2):
                        nc.tensor.transpose(
                            pb[:, 0, lt * LC : (lt + 1) * LC],
                            lng[:LC, lt, st, dh * 128 : (dh + 1) * 128], c["eyeb"])
                    nc.scalar.activation(out=lgT[:, st, dh, :], in_=pb[:, 0, :],
                                         func=AF.Identity, bias=0.0, scale=1.0)
                    pf = pst.tile([128, 2, L], F32, name=f"pf1_{g}_{st}{dh}", tag="pTf")
                    for lt in range(2):
                        nc.tensor.transpose(
                            pf[:, 0, lt * LC : (lt + 1) * LC],
                            pa[:LC, lt, st, dh * 128 : (dh + 1) * 128], c["eyef"])
                    nc.scalar.activation(out=khT[:, st, dh, :], in_=pf[:, 0, :],
                                         func=AF.Identity, bias=c["b4"][:, dh : dh + 1],
                                         scale=c["g4"][:, dh : dh + 1])
        if upto <= 5:
            continue

        # ---- W matmul (fp32r, stream-pairs) + relu -> zr ----
        zrs = {}
        for pi, b in enumerate(seqs):
            st0 = 2 * b
            zr = op.tile([128, 2, 2, L], F32R, name=f"zr_{g}_{pi}", tag="zr", bufs=3)
            for do in range(2):
                pw = psw.tile([128, 2, L], F32, name=f"pw_{g}_{pi}{do}", tag="pw")
                for di in range(2):
                    nc.tensor.matmul(
                        pw, c["wwt"][:, di, do, :],
                        khT[:, st0 : st0 + 2, di, :],
                        start=(di == 0), stop=(di == 1))
                nc.scalar.activation(out=zr[:, :, do, :], in_=pw, func=AF.Relu,
                                     bias=c["wb"][:, do : do + 1], scale=1.0)
            zrs[b] = zr
        if upto <= 6:
            continue

        # ---- w logits as FD=1 column matmuls (keyv add via PSUM accum) ----
        wc = _pA(f"wcol_{g}")
        for si, st in enumerate(sts):
            b, br = st // 2, st % 2
            zr = zrs[b]
            zi = br
            for lc in range(2):
                lsl = slice(lc * LC, (lc + 1) * LC)
                outc = wc[:LC, lc, si : si + 1]
                mms = [khT[:, st, 0, lsl].bitcast(F32), khT[:, st, 1, lsl].bitcast(F32),
                       zr[:, zi, 0, lsl].bitcast(F32), zr[:, zi, 1, lsl].bitcast(F32)]
                for i, sta in enumerate(mms):
                    nc.tensor.matmul(outc, sta, qT[:, i % 2, br, b : b + 1],
                                     start=(i == 0), stop=(i == 3))
        # ---- w softmax (column form) ----
        ns_g = 2 * sgrp
        nc.scalar.activation(out=wex[:, :, gssl], in_=wc[:LC, :, :ns_g],
                             func=AF.Exp, bias=0.0, scale=SCALE)
        pws = _pA(f"pws_{g}")
        pwsum = pws[0:1, 0, :ns_g]
        for lc in range(2):
            nc.tensor.matmul(pwsum, c["onesc"], wex[:, lc, gssl],
                             start=(lc == 0), stop=(lc == 1))
        wrc = tmp.tile([1, ns_g], F32, name=f"wrc_{g}", tag="wrc", bufs=2)
        nc.vector.reciprocal(wrc, pwsum)
        pwbt = _pA(f"pwb_{g}")
        pwb = pwbt[:LC, 0, :ns_g]
        nc.tensor.matmul(pwb, c["onesr"], wrc, start=True, stop=True)
        nc.vector.tensor_tensor(
            out=wn[:, :, gssl], in0=wex[:, :, gssl],
            in1=bass.AP(tensor=pwb.tensor, offset=pwb.offset,
                        ap=[list(pwb.ap[0]), [0, 2], list(pwb.ap[1])]),
            op=OP.mult)
        if upto <= 8:
            continue

        # ---- score matmuls + sex = exp(SCALE*rstd1*logit) * c2f ----
        for st in sts:
            for lt in range(2):
                psSt = _pA(f"psS_{g}_{st}{lt}")
                psS = psSt[:LC, 0, :K]
                for dh in range(2):
                    nc.tensor.matmul(psS, lgT[:, st, dh, lt * LC : (lt + 1) * LC],
                                     c["m2tb"][:, dh, :], start=(dh == 0), stop=(dh == 1))
                nc.scalar.activation(out=sex[:LC, lt, st, :], in_=psS, func=AF.Exp,
                                     bias=0.0, scale=rs1s[:LC, lt, st, 0:1])
        # batched alpha chain on [:LC]
        c2b = _bins(_bins(c["c2fb"][:LC, :], 1, 2), 2, 2 * sgrp)
        for ssl in (gssl,):
            sexs = sex[:LC, :, ssl, :]
            ccas = cca[:LC, :, ssl, :]
            nc.vector.tensor_tensor(out=sexs, in0=sexs, in1=c2b, op=OP.mult)
            ssm = tmp.tile([LC, 2, 2 * sgrp, 1], F32, name=f"ssm_{g}", tag="ssm", bufs=2)
            nc.vector.reduce_sum(out=ssm, in_=sexs, axis=AX)
            src_ = tmp.tile([LC, 2, 2 * sgrp, 1], F32, name=f"src_{g}", tag="src", bufs=2)
            nc.vector.reciprocal(src_, ssm)
            nc.vector.tensor_tensor(out=ccas, in0=sexs, in1=_b0(src_[:, :, :, 0:1]),
                                    op=OP.mult)
            nc.vector.tensor_tensor(out=ccas, in0=ccas, in1=_b0a(wn[:, :, ssl]),
                                    op=OP.mult)
            sq = tmp.tile([LC, 2, 2 * sgrp, K], F32, name=f"sq_{g}", tag="sq", bufs=2)
            nc.vector.tensor_mul(sq, ccas, ccas)
            nc.vector.tensor_tensor(out=sq, in0=sq,
                                    in1=_b0(mv1[:LC, :, ssl, 1:2]), op=OP.mult)
            # (kept as two ops: stt scalar slot needs a single column)
            nc.scalar.activation(out=sq, in_=sq, func=AF.Ln, bias=c["epsc"][:LC, :],
                                 scale=1.0)
            nc.scalar.activation(out=sq, in_=sq, func=AF.Exp, bias=0.0, scale=-0.5)
            nc.vector.tensor_tensor(out=alf[:LC, :, ssl, :], in0=ccas, in1=sq,
                                    op=OP.mult)
        if upto <= 9:
            continue

        # ---- final: PE diag matmuls -> PSUM -> copy/stt -> SBUF -> DMA ----
        n_pe = K - n_hyb
        for b in seqs:
            stL, stG = 2 * b, 2 * b + 1
            for lt in range(2):
                osb = op.tile([LC, K, D], F32, name=f"osb_{b}_{lt}", tag="osb", bufs=3)
                for kc in range(0, n_pe, 2):
                    p10 = _pA(f"p10_{b}_{lt}{kc}")[:LC]
                    for ki in range(2):
                        k = kc + ki
                        dgL = tmp.tile([LC + 1, LC], BF16, name=f"dgL_{b}_{lt}{k}",
                                       tag="dg", bufs=6)
                        _diag(dgL, c["eyex1"], alf[: LC + 1, lt, stL, k : k + 1])
                        dgG = tmp.tile([LC + 1, LC], BF16, name=f"dgG_{b}_{lt}{k}",
                                       tag="dg", bufs=6)
                        _diag(dgG, c["eyex1"], alf[: LC + 1, lt, stG, k : k + 1])
                        nc.tensor.matmul(p10[:, ki, :], dgL, lng[: LC + 1, lt, stL, :],
                                         start=True, stop=False)
                        nc.tensor.matmul(p10[:, ki, :], dgG, lng[: LC + 1, lt, stG, :],
                                         start=False, stop=True)
                    _copy10(osb[:, kc : kc + 2, :], p10)
                p1ht = _pA(f"p1h_{b}_{lt}") if n_hyb else None
                for k in range(n_pe, K):
                    p1h = p1ht[:LC, k - n_pe, :]
                    dgG = tmp.tile([LC + 1, LC], BF16, name=f"dgGh_{b}_{lt}{k}",
                                   tag="dg", bufs=6)
                    _diag(dgG, c["eyex2"], alf[: LC + 1, lt, stG, k : k + 1])
                    nc.tensor.matmul(p1h, dgG, lng[: LC + 1, lt, stG, :],
                                     start=True, stop=True)
                    nc.vector.scalar_tensor_tensor(
                        out=osb[:, k, :], in0=lng[:LC, lt, stL, :],
                        scalar=alf[:LC, lt, stL, k : k + 1], in1=p1h,
                        op0=OP.mult, op1=OP.add)
                if dma_out:
                    nc.sync.dma_start(out=_dram_out_ap(out_t, b, lt), in_=osb)
                else:
                    nc.sync.dma_start(out=bass.AP(tensor=out_t, offset=0,
                                                  ap=[[D, 1], [1, 4]]),
                                      in_=osb[0:1, 0, 0:4])


def build_module(reps=1, upto=12, timing=False, n_hyb=2, diag_eng="dve", sgrp=2,
                 dma_out=True, diag_mix=("dve",), copy_mix=("act",), pair_t=False,
                 pbufs=(1, 2, 4)):
    nc = bacc.Bacc("TRN2", target_bir_lowering=False, debug=False,
                   num_devices=NCORES)
    big = "Internal" if timing else "ExternalInput"
    t = {}
    t["xL"] = nc.dram_tensor("xL", [BPC, L, D], F32, kind=big)
    t["xG"] = nc.dram_tensor("xG", [BPC, L, D], F32, kind=big)
    t["bf32"] = nc.dram_tensor("bf32", [128, CF32], F32, kind="ExternalInput")
    t["bb16"] = nc.dram_tensor("bb16", [128, CB16], BF16, kind="ExternalInput")
    t["wwt"] = nc.dram_tensor("wwt", [D, D], F32R, kind="ExternalInput")
    t["b5rep"] = nc.dram_tensor("b5rep", [5, 2, NS, D], BF16, kind="ExternalInput")
    t["qraw"] = nc.dram_tensor("qraw", [BPC, 2, D], F32, kind="ExternalInput")
    t["posrou"] = nc.dram_tensor("posrou", [BPC, D], F32, kind="ExternalInput")
    out_t = nc.dram_tensor("out", [BPC, K, L, D], F32,
                           kind="Internal" if timing else "ExternalOutput")
    sink_t = None
    if timing:
        sink_t = nc.dram_tensor("sink", [1, 4], F32, kind="ExternalOutput")

    with tile.TileContext(nc) as tc:
        with ExitStack() as ctx:
            cst = _emit_consts(nc, tc, ctx, t)
            blk = ctx.enter_context(tc.tile_pool(name="blk", bufs=1))
            tmp = ctx.enter_context(tc.tile_pool(name="tmp", bufs=2))
            pst = ctx.enter_context(tc.tile_pool(name="pst", bufs=pbufs[0], space="PSUM"))
            psw = ctx.enter_context(tc.tile_pool(name="psw", bufs=pbufs[1], space="PSUM"))
            psA = ctx.enter_context(tc.tile_pool(name="psA", bufs=pbufs[2], space="PSUM"))
            op = ctx.enter_context(tc.tile_pool(name="outp", bufs=1))
            pools = (blk, tmp, pst, psw, psA, op)
            if reps == 1:
                _emit_body(nc, tc, cst, pools, t, out_t, upto, n_hyb, diag_eng,
                           sgrp, dma_out, diag_mix, copy_mix, pair_t)
            else:
                with tc.For_i(0, reps, 1):
                    _emit_body(nc, tc, cst, pools, t, out_t, upto, n_hyb,
                               diag_eng, sgrp, dma_out, diag_mix, copy_mix, pair_t)
            if sink_t is not None:
                snk = tmp.tile([1, 4], F32, name="snk", tag="snk", bufs=1)
                nc.sync.dma_start(out=snk, in_=out_t[0, 0, 0:1, 0:4])
                nc.sync.dma_start(out=sink_t[:, :], in_=snk)
    nc.compile()
    return nc


def host_inputs(local_item_emb, global_item_emb, intentions, pos_fai, rou, W_w, W_b,
                g1, b1, g2, b2, g3, b3, g4, b4, g5, b5, seq_len):
    """Host-side param folding + per-core sharding. Returns in_maps list."""
    import ml_dtypes
    f = np.float32
    bf = ml_dtypes.bfloat16
    xL = np.ascontiguousarray(local_item_emb, f)
    xG = np.ascontiguousarray(global_item_emb, f)
    g1, b1, g2, b2 = (np.asarray(v, f) for v in (g1, b1, g2, b2))
    g3, b3, g4, b4 = (np.asarray(v, f) for v in (g3, b3, g4, b4))
    g5, b5 = np.asarray(g5, f), np.asarray(b5, f)
    intentions = np.asarray(intentions, f)
    mu = intentions.mean(-1, keepdims=True)
    var = ((intentions - mu) ** 2).mean(-1, keepdims=True)
    ln2 = (intentions - mu) / np.sqrt(var + EPS) * g2 + b2          # [K, D]
    assert np.abs(g5).min() > 1e-3, "g5 too small for m2 folding"
    m2 = np.ascontiguousarray((ln2 * (g1 / g5)[None, :]).T, f)      # [D, K]
    c2 = (ln2 @ b1.astype(np.float64)).astype(f).reshape(1, K)
    c2f = np.exp(SCALE * c2).astype(f)
    eye = np.eye(LC, dtype=f)
    eyex1 = np.concatenate([eye, np.ones((1, LC), f)], 0)
    eyex2 = np.concatenate([eye, 2.0 * np.ones((1, LC), f)], 0)
    b5rep = np.ascontiguousarray(np.broadcast_to(b5.reshape(1, 1, 1, D), (5, 2, NS, D))).astype(bf)
    sl = np.asarray(seq_len).astype(np.int64).reshape(B)
    idx = sl - 1
    posrou_all = (np.asarray(pos_fai, f)[idx] + np.asarray(rou, f)[None, :])  # [B, D]
    pos = np.asarray(pos_fai, f)
    named = {
        "pos": pos.reshape(2, LC, D).transpose(1, 0, 2),
        "g5bc": np.broadcast_to(g5.reshape(1, D), (LC, D)),
        "c2fb": np.broadcast_to(c2f, (LC, K)),
        "g3bc": np.broadcast_to(g3.reshape(1, D), (BPC, D)),
        "b3bc": np.broadcast_to(b3.reshape(1, D), (BPC, D)),
        "g4": g4.reshape(2, 128).T, "b4": b4.reshape(2, 128).T,
        "wb": np.asarray(W_b, f).reshape(2, 128).T,
        "eyef": eye,
        "m2tb": m2.reshape(2, 128, K).transpose(1, 0, 2),
        "eyeb": eye, "eyex1": eyex1, "eyex2": eyex2,
    }
    bf32_blob, bb16_blob = _pack_consts(named)
    shared = {
        "bf32": bf32_blob, "bb16": bb16_blob,
        "wwt": np.ascontiguousarray(np.asarray(W_w, f).T),
        "b5rep": b5rep,
    }
    in_maps = []
    for cix in range(NCORES):
        s = slice(cix * BPC, (cix + 1) * BPC)
        qraw = np.stack([xL[s][np.arange(BPC), idx[s]],
                         xG[s][np.arange(BPC), idx[s]]], axis=1)     # [BPC, 2, D]
        in_maps.append({"xL": xL[s], "xG": xG[s],
                        "qraw": np.ascontiguousarray(qraw, f),
                        "posrou": np.ascontiguousarray(posrou_all[s], f),
                        **shared})
    return in_maps


_module_cache = {}


def kernel(**inputs) -> np.ndarray:
    in_maps = host_inputs(**inputs)
    if 1 not in _module_cache:
        _module_cache[1] = build_module(reps=1)
    nc = _module_cache[1]
    r = run_bass_kernel_spmd(nc, in_maps, list(range(NCORES)))
    out = np.concatenate([r.results[cix]["out"] for cix in range(NCORES)], axis=0)
    return out.astype(np.float32)


# revision 4
# speedup vs baseline: 1.1323x; 1.1323x over previous
"""Trainium2 Bass kernel v2 for nn_DisentangleEncoder (B=64, L=200, D=256, K=8).

Data-parallel over batch: 8 sequences per NeuronCore x 8 cores.

Key math (per branch, per sequence b, x = item_emb[b]):
  mu1/var1  = stats of x over D
  lga       = (x - mu1) * g5                  [NO rstd1 -- folded elsewhere]
  score     = softmax_K( exp(SCALE*rstd1*(lgaT @ m2)) * c2f )
              with m2[d,k] = ln2[k,d]*g1[d]/g5[d], c2f[k] = exp(SCALE*(b1.ln2[k]))
  pa        = x + pos;  xpfn = (pa - mu4) * rstd4
  khT       = transpose(xpfn)*g4 + b4         [D on partitions]
  zr        = relu(W_w @ khT + W_b)
  wlogit[l] = q . khT[:,l] + q . zr[:,l]      (keyv add folded into PE accum)
  q         = LN(x[b,sl-1] + pos[sl-1] + rou)*g3 + b3   (gathers done on host)
  w         = softmax_L(wlogit * SCALE)
  cc[k,l]   = score[l,k] * w[l]
  alpha     = cc / sqrt(cc^2*var1 + eps)      [exact: LN(c*x) = alpha*lga + b5]
Final: out[k,l,:] = alphaL[k,l]*lgaL[l,:] + alphaG[k,l]*lgaG[l,:] + 2*b5
  computed on the PE as diag(alpha) matmuls in bf16 accumulated in PSUM;
  the 2*b5 term rides along as a 101st contraction row (ones in the diag
  stationary, b5 replicated in partition 100 of the lng tile).

All sqrt/rsqrt via exp(+-0.5*ln(v)) so ACT needs one table set.
"""

import numpy as np
from contextlib import ExitStack

import concourse.bacc as bacc
import concourse.bass as bass
import concourse.tile as tile
from concourse import mybir
from concourse.bass_utils import run_bass_kernel_spmd

# Route all Exp/Ln activations to the single `natural_log_exp_and_others`
# table set (one ACT_TABLE_LOAD instead of ~2.7us per switch).
_orig_gat = bacc.get_activation_tables


def _gat_joint_exp_ln(arch):
    tabs = _orig_gat(arch)
    out = {}
    for name, fns in tabs.items():
        fns = set(fns)
        if name != "natural_log_exp_and_others":
            fns.discard(mybir.ActivationFunctionType.Exp)
            fns.discard(mybir.ActivationFunctionType.Ln)
        out[name] = fns
    return out


bacc.get_activation_tables = _gat_joint_exp_ln

B, L, D, K = 64, 200, 256, 8
NCORES = 8
BPC = B // NCORES
EPS = 1e-5
SCALE = 1.0 / float(np.sqrt(D))
F32 = mybir.dt.float32
F32R = mybir.dt.float32r
BF16 = mybir.dt.bfloat16
LC = 100                  # l-chunk size; L = 2*LC
NS = 2 * BPC              # streams per core (seq x branch)
AX = mybir.AxisListType.X
OP = mybir.AluOpType
AF = mybir.ActivationFunctionType


def _bc(ap, p):
    """Broadcast a DRAM AP across p partitions (partition-step 0)."""
    return bass.AP(tensor=ap.tensor, offset=ap.offset, ap=[[0, p]] + list(ap.ap))


def _b0(ap, n=K):
    """Replace the (size-1) last axis of an AP with a step-0 broadcast of n."""
    a = list(ap.ap)
    a[-1] = [0, n]
    return bass.AP(tensor=ap.tensor, offset=ap.offset, ap=a)


def _b0a(ap, n=K):
    """Append a step-0 broadcast axis of size n to an AP."""
    a = list(ap.ap) + [[0, n]]
    return bass.AP(tensor=ap.tensor, offset=ap.offset, ap=a)


def _bins(ap, pos, n):
    """Insert a step-0 broadcast axis of size n at free-dim position pos."""
    a = list(ap.ap)
    a.insert(pos, [0, n])
    return bass.AP(tensor=ap.tensor, offset=ap.offset, ap=a)




# (name, free-offset, free-shape, partitions) inside the packed const blobs
F32_LAYOUT = [
    ("pos",    0,           (2, D), LC),    # [LC, 2, D]
    ("g5bc",   2 * D,       (D,),   LC),
    ("c2fb",   3 * D,       (K,),   LC),
    ("g3bc",   3 * D + K,   (D,),   BPC),
    ("b3bc",   4 * D + K,   (D,),   BPC),
    ("g4",     5 * D + K,   (2,),   128),
    ("b4",     5 * D + K + 2, (2,), 128),
    ("wb",     5 * D + K + 4, (2,), 128),
    ("eyef",   5 * D + K + 6, (LC,), LC),
]
CF32 = 5 * D + K + 6 + LC
B16_LAYOUT = [
    ("m2tb",  0,            (2, K), 128),
    ("eyeb",  2 * K,        (LC,),  LC),
    ("eyex1", 2 * K + LC,   (LC,),  LC + 1),
    ("eyex2", 2 * K + 2 * LC, (LC,), LC + 1),
]
CB16 = 2 * K + 3 * LC


def _pack_consts(shared_np):
    """Build the two const blobs from a dict of named numpy arrays.
    Each entry is [p, *shape]; rows beyond p are left zero."""
    import ml_dtypes
    bf32 = np.zeros((128, CF32), np.float32)
    bb16 = np.zeros((128, CB16), ml_dtypes.bfloat16)
    for nm, off, shape, p in F32_LAYOUT:
        v = shared_np[nm].reshape(p, int(np.prod(shape)))
        bf32[:p, off : off + v.shape[1]] = v
    for nm, off, shape, p in B16_LAYOUT:
        v = shared_np[nm].reshape(p, int(np.prod(shape)))
        bb16[:p, off : off + v.shape[1]] = v
    return bf32, bb16

def _dram_x_ap(t, b):
    """DRAM AP for x[b] viewed as [LC part, 2 lt, D]."""
    return bass.AP(tensor=t, offset=b * L * D, ap=[[D, LC], [LC * D, 2], [1, D]])


def _dram_out_ap(t, b, lt):
    """DRAM AP for out[b, :, lt*LC:(lt+1)*LC, :] viewed as [LC part, K, D]."""
    return bass.AP(tensor=t, offset=b * K * L * D + lt * LC * D,
                   ap=[[D, LC], [L * D, K], [1, D]])


def _emit_consts(nc, tc, ctx, t):
    """All f32 consts live in one packed [128, CF32] blob (one DMA), all bf16
    consts in one [128, CB16] blob, wwt (f32r) in its own tensor. Entries in
    the returned dict are AP slices of the blobs."""
    cp = ctx.enter_context(tc.tile_pool(name="consts", bufs=1))
    c = {}
    bf32 = cp.tile([128, CF32], F32, name="c_bf32")
    nc.sync.dma_start(out=bf32, in_=t["bf32"][:, :])
    bb16 = cp.tile([128, CB16], BF16, name="c_bb16")
    nc.sync.dma_start(out=bb16, in_=t["bb16"][:, :])
    wwt = cp.tile([128, 2, 2, 128], F32R, name="c_wwt")
    nc.sync.dma_start(out=wwt, in_=bass.AP(tensor=t["wwt"], offset=0,
                                           ap=[[D, 128], [128 * D, 2], [128, 2], [1, 128]]))
    c["wwt"] = wwt

    def _sl(blob, off, shape, p):
        stride = 1; rev = []
        for s in reversed(shape):
            rev.append([stride, s]); stride *= s
        dims = list(reversed(rev))
        return bass.AP(tensor=blob.tensor, offset=blob.offset + off,
                       ap=[[blob.ap[0][0], p]] + dims)

    def f32sl(off, shape, p=128):
        return _sl(bf32, off, shape, p)

    def b16sl(off, shape, p=128):
        return _sl(bb16, off, shape, p)

    for nm, off, shape, p in F32_LAYOUT:
        c[nm] = f32sl(off, shape, p)
    for nm, off, shape, p in B16_LAYOUT:
        c[nm] = b16sl(off, shape, p)
    qraw = cp.tile([BPC, 2, D], F32, name="c_qraw")
    nc.sync.dma_start(out=qraw, in_=t["qraw"][:, :, :])
    c["qraw"] = qraw
    posrou = cp.tile([BPC, D], F32, name="c_posrou")
    nc.sync.dma_start(out=posrou, in_=t["posrou"][:, :])
    c["posrou"] = posrou
    epsc = cp.tile([128, 1], F32, name="c_epsc")
    nc.gpsimd.memset(epsc, EPS)
    c["epsc"] = epsc
    lnsc = cp.tile([128, 1], F32, name="c_lnsc")
    nc.gpsimd.memset(lnsc, float(np.log(SCALE)))
    c["lnsc"] = lnsc
    onesc = cp.tile([LC, 1], F32, name="c_onesc")
    nc.gpsimd.memset(onesc, 1.0)
    c["onesc"] = onesc
    rc256 = cp.tile([LC, 1], F32, name="c_rc256")
    nc.gpsimd.memset(rc256, 1.0 / D)
    c["rc256"] = rc256
    onesr = cp.tile([1, LC], F32, name="c_onesr")
    nc.gpsimd.memset(onesr, 1.0)
    c["onesr"] = onesr
    return c


def _emit_body(nc, tc, c, pools, t, out_t, upto=12, n_hyb=2, diag_eng="gp",
               sgrp=2, dma_out=True, diag_mix=None, copy_mix=None, pair_t=True,
               batch_diag=False, dma_split=1, deep=False, hyb_first=False):
    """Emit the whole per-core computation.

    n_hyb:    number of k's (taken from the top) routed through the hybrid
              path (PE does the G-branch matmul, DVE stt fuses the L-branch
              multiply-add + PSUM read + SBUF write).
    diag_eng: 'gp' | 'dve' — engine that builds diag(alpha) stationaries.
    sgrp:     seqs per pipeline group (BPC % sgrp == 0).
    """
    blk, tmp, pst, psw, psA, op = pools

    def _pA(name):
        return psA.tile([128, 2, D], F32, name=name, tag="pA")
    xsrc = {0: t["xL"], 1: t["xG"]}
    # rotation lists: engines picked round-robin for diag builds / p10 copies
    if diag_mix is None:
        diag_mix = {"gp": (nc.gpsimd,), "dve": (nc.vector,)}[diag_eng]
    else:
        diag_mix = tuple({"gp": nc.gpsimd, "dve": nc.vector, "act": "act"}[e]
                         for e in diag_mix)
    if copy_mix is None:
        copy_mix = ("act", "act", "dve")
    _dcnt = [0]
    def _diag(dg, eyex, col):
        e = diag_mix[_dcnt[0] % len(diag_mix)]; _dcnt[0] += 1
        if e == "act":
            nc.scalar.activation(out=dg, in_=eyex, func=AF.Identity, bias=0.0,
                                 scale=col)
        elif e is nc.gpsimd:
            nc.gpsimd.tensor_tensor(out=dg, in0=eyex, in1=_b0(col, LC),
                                    op=OP.mult)
        else:
            e.tensor_scalar_mul(dg, eyex, col)
    _ccnt = [0]
    def _copy10(dst, srcp):
        e = copy_mix[_ccnt[0] % len(copy_mix)]; _ccnt[0] += 1
        if e == "act":
            nc.scalar.activation(out=dst, in_=srcp, func=AF.Identity, bias=0.0,
                                 scale=1.0)
        else:
            nc.vector.tensor_copy(dst, srcp)

    # ---- persistent tiles ----
    xa = blk.tile([128, 2, NS, D], F32, name="xa", tag="xa")
    pa = blk.tile([128, 2, NS, D], F32, name="pa", tag="pa")
    lng = blk.tile([128, 2, NS, D], BF16, name="lng", tag="lng")
    lgT = blk.tile([128, NS, 2, L], BF16, name="lgT", tag="lgT")
    khT = blk.tile([128, NS, 2, L], F32R, name="khT", tag="khT")
    mv1 = blk.tile([128, 2, NS, 2], F32, name="mv1", tag="mv1")
    mv4 = blk.tile([128, 2, NS, 2], F32, name="mv4", tag="mv4")
    rs1s = blk.tile([128, 2, NS, 1], F32, name="rs1s", tag="rs1s")
    rs4 = blk.tile([128, 2, NS, 1], F32, name="rs4", tag="rs4")
    sex = blk.tile([128, 2, NS, K], F32, name="sex", tag="sex")
    cca = blk.tile([128, 2, NS, K], F32, name="cca", tag="cca")
    alf = blk.tile([128, 2, NS, K], F32, name="alf", tag="alf")
    wex = blk.tile([LC, 2, NS], F32, name="wex", tag="wex")
    wn = blk.tile([LC, 2, NS], F32, name="wn", tag="wn")
    qp = blk.tile([BPC, 2, D], F32, name="qp", tag="qp")
    qT = blk.tile([128, 2, 2, BPC], F32, name="qT", tag="qT")

    # partition LC of lng = b5 row (per (lt,st) free offset); alpha row LC = 1.0
    nc.sync.dma_start(out=lng[96 : LC + 1, :, :, :], in_=t["b5rep"][:, :, :, :])
    nc.gpsimd.memset(alf[96:128, :, :, :], 1.0)

    # ---- q chain (global; tiny) ----
    nc.vector.tensor_tensor(
        out=qp, in0=c["qraw"], in1=_bins(c["posrou"], 1, 2), op=OP.add)
    q6 = tmp.tile([BPC, 2, 6], F32, name="q6", tag="q6", bufs=1)
    qmv = tmp.tile([BPC, 2, 2], F32, name="qmv", tag="qmv", bufs=1)
    for br in range(2):
        nc.vector.bn_stats(out=q6[:, br, :], in_=qp[:, br, :])
        nc.vector.bn_aggr(out=qmv[:, br, :], in_=q6[:, br, :])
    qle = tmp.tile([BPC, 2, 1], F32, name="qle", tag="qle", bufs=1)
    nc.scalar.activation(out=qle, in_=qmv[:, :, 1:2], func=AF.Ln,
                         bias=c["epsc"][:BPC, :], scale=1.0)
    qrs = tmp.tile([BPC, 2, 1], F32, name="qrs", tag="qrs", bufs=1)
    nc.scalar.activation(out=qrs, in_=qle, func=AF.Exp, bias=0.0, scale=-0.5)
    for br in range(2):
        nc.vector.tensor_scalar(qp[:, br, :], qp[:, br, :], qmv[:, br, 0:1],
                                qrs[:, br, 0:1], OP.subtract, OP.mult)
        nc.vector.tensor_mul(qp[:, br, :], qp[:, br, :], c["g3bc"])
        nc.vector.tensor_add(qp[:, br, :], qp[:, br, :], c["b3bc"])
        for dh in range(2):
            pqt = _pA(f"pq{br}{dh}")
            pq = pqt[:, 0, :BPC]
            nc.tensor.transpose(pq, qp[:, br, dh * 128 : (dh + 1) * 128],
                                c["eyef"][:BPC, :BPC])
            nc.scalar.activation(out=qT[:, dh, br, :], in_=pq, func=AF.Identity,
                                 bias=0.0, scale=1.0)

    # ---- all x loads up front (SP starts immediately; consts follow) ----
    for st in range(NS):
        b, br = st // 2, st % 2
        nc.sync.dma_start(out=xa[:LC, :, st, :], in_=_dram_x_ap(xsrc[br], b))

    ngrp = BPC // sgrp
    for g in range(ngrp):
        seqs = list(range(g * sgrp, (g + 1) * sgrp))
        sts = [2 * b + br for b in seqs for br in range(2)]
        gs0, gs1 = 2 * sgrp * g, 2 * sgrp * (g + 1)
        gssl = slice(gs0, gs1)               # contiguous group stream slice

        # ---- x stats (DVE) + pa with accumulated moments (GP) ----
        for st in sts:
            for lt in range(2):
                s6 = tmp.tile([LC, 6], F32, name=f"s6_{g}_{st}{lt}", tag="s6", bufs=4)
                nc.vector.bn_stats(out=s6, in_=xa[:LC, lt, st, :])
                nc.vector.bn_aggr(out=mv1[:LC, lt, st, :], in_=s6)
        for st in sts:
            for lt in range(2):
                nc.gpsimd.tensor_add(pa[:LC, lt, st, :], xa[:LC, lt, st, :],
                                     c["pos"][:, lt, :])
                s6 = tmp.tile([LC, 6], F32, name=f"s64_{g}_{st}{lt}", tag="s6",
                              bufs=4)
                nc.vector.bn_stats(out=s6, in_=pa[:LC, lt, st, :])
                nc.vector.bn_aggr(out=mv4[:LC, lt, st, :], in_=s6)
        # ---- rstd chains (batched per group) ----
        ns_g = 2 * sgrp
        for mv, rs, bias in ((mv1, rs1s, c["lnsc"]), (mv4, rs4, 0.0)):
            le = tmp.tile([LC, 2, ns_g, 1], F32, name=f"le_{g}", tag="le", bufs=2)
            nc.scalar.activation(out=le, in_=mv[:LC, :, gssl, 1:2], func=AF.Ln,
                                 bias=c["epsc"][:LC, :], scale=1.0)
            if isinstance(bias, float):
                nc.scalar.activation(out=rs[:LC, :, gssl, :], in_=le,
                                     func=AF.Exp, bias=bias, scale=-0.5)
            else:
                nc.scalar.activation(out=rs[:LC, :, gssl, :], in_=le,
                                     func=AF.Exp, bias=bias[:LC, :], scale=-0.5)
        # ---- lga (bf16) + xpfn in place ----
        for st in sts:
            for lt in range(2):
                nc.vector.scalar_tensor_tensor(
                    out=lng[:LC, lt, st, :], in0=xa[:LC, lt, st, :],
                    scalar=mv1[:LC, lt, st, 0:1], in1=c["g5bc"],
                    op0=OP.subtract, op1=OP.mult)
                nc.vector.tensor_scalar(
                    pa[:LC, lt, st, :], pa[:LC, lt, st, :],
                    mv4[:LC, lt, st, 0:1], rs4[:LC, lt, st, 0:1],
                    OP.subtract, OP.mult)
        if upto <= 4:
            continue

        # ---- transposes: lng -> lgT (bf16), xpfn -> khT (f32, g4/b4) ----
        if pair_t:
            for b in seqs:
                st0 = 2 * b
                for dh in range(2):
                    pb = pst.tile([128, 2, L], BF16, name=f"pb_{g}_{b}{dh}", tag="pTb")
                    for si in range(2):
                        for lt in range(2):
                            nc.tensor.transpose(
                                pb[:, si, lt * LC : (lt + 1) * LC],
                                lng[:LC, lt, st0 + si, dh * 128 : (dh + 1) * 128],
                                c["eyeb"])
                    nc.scalar.activation(out=lgT[:, st0 : st0 + 2, dh, :], in_=pb,
                                         func=AF.Identity, bias=0.0, scale=1.0)
                    pf = pst.tile([128, 2, L], F32, name=f"pf_{g}_{b}{dh}", tag="pTf")
                    for si in range(2):
                        for lt in range(2):
                            nc.tensor.transpose(
                                pf[:, si, lt * LC : (lt + 1) * LC],
                                pa[:LC, lt, st0 + si, dh * 128 : (dh + 1) * 128],
                                c["eyef"])
                    nc.scalar.activation(out=khT[:, st0 : st0 + 2, dh, :], in_=pf,
                                         func=AF.Identity, bias=c["b4"][:, dh : dh + 1],
                                         scale=c["g4"][:, dh : dh + 1])
        else:
            for st in sts:
                for dh in range(2):
                    pb = pst.tile([128, 2, L], BF16, name=f"pb1_{g}_{st}{dh}", tag="pTb")
                    for lt in range(2):
                        nc.tensor.transpose(
                            pb[:, 0, lt * LC : (lt + 1) * LC],
                            lng[:LC, lt, st, dh * 128 : (dh + 1) * 128], c["eyeb"])
                    nc.scalar.activation(out=lgT[:, st, dh, :], in_=pb[:, 0, :],
                                         func=AF.Identity, bias=0.0, scale=1.0)
                    pf = pst.tile([128, 2, L], F32, name=f"pf1_{g}_{st}{dh}", tag="pTf")
                    for lt in range(2):
                        nc.tensor.transpose(
                            pf[:, 0, lt * LC : (lt + 1) * LC],
                            pa[:LC, lt, st, dh * 128 : (dh + 1) * 128], c["eyef"])
                    nc.scalar.activation(out=khT[:, st, dh, :], in_=pf[:, 0, :],
                                         func=AF.Identity, bias=c["b4"][:, dh : dh + 1],
                                         scale=c["g4"][:, dh : dh + 1])
        if upto <= 5:
            continue

        # ---- W matmul (fp32r, stream-pairs) + relu -> zr ----
        zrs = {}
        for pi, b in enumerate(seqs):
            st0 = 2 * b
            zr = op.tile([128, 2, 2, L], F32R, name=f"zr_{g}_{pi}", tag="zr", bufs=3)
            for do in range(2):
                pw = psw.tile([128, 2, L], F32, name=f"pw_{g}_{pi}{do}", tag="pw")
                for di in range(2):
                    nc.tensor.matmul(
                        pw, c["wwt"][:, di, do, :],
                        khT[:, st0 : st0 + 2, di, :],
                        start=(di == 0), stop=(di == 1))
                nc.scalar.activation(out=zr[:, :, do, :], in_=pw, func=AF.Relu,
                                     bias=c["wb"][:, do : do + 1], scale=1.0)
            zrs[b] = zr
        if upto <= 6:
            continue

        # ---- w logits as FD=1 column matmuls (keyv add via PSUM accum) ----
        wc = _pA(f"wcol_{g}")
        for si, st in enumerate(sts):
            b, br = st // 2, st % 2
            zr = zrs[b]
            zi = br
            for lc in range(2):
                lsl = slice(lc * LC, (lc + 1) * LC)
                outc = wc[:LC, lc, si : si + 1]
                mms = [khT[:, st, 0, lsl].bitcast(F32), khT[:, st, 1, lsl].bitcast(F32),
                       zr[:, zi, 0, lsl].bitcast(F32), zr[:, zi, 1, lsl].bitcast(F32)]
                for i, sta in enumerate(mms):
                    nc.tensor.matmul(outc, sta, qT[:, i % 2, br, b : b + 1],
                                     start=(i == 0), stop=(i == 3))
        # ---- w softmax (column form) ----
        ns_g = 2 * sgrp
        nc.scalar.activation(out=wex[:, :, gssl], in_=wc[:LC, :, :ns_g],
                             func=AF.Exp, bias=0.0, scale=SCALE)
        pws = _pA(f"pws_{g}")
        pwsum = pws[0:1, 0, :ns_g]
        for lc in range(2):
            nc.tensor.matmul(pwsum, c["onesc"], wex[:, lc, gssl],
                             start=(lc == 0), stop=(lc == 1))
        wrc = tmp.tile([1, ns_g], F32, name=f"wrc_{g}", tag="wrc", bufs=2)
        nc.vector.reciprocal(wrc, pwsum)
        pwbt = _pA(f"pwb_{g}")
        pwb = pwbt[:LC, 0, :ns_g]
        nc.tensor.matmul(pwb, c["onesr"], wrc, start=True, stop=True)
        nc.vector.tensor_tensor(
            out=wn[:, :, gssl], in0=wex[:, :, gssl],
            in1=bass.AP(tensor=pwb.tensor, offset=pwb.offset,
                        ap=[list(pwb.ap[0]), [0, 2], list(pwb.ap[1])]),
            op=OP.mult)
        if upto <= 8:
            continue

        # ---- score matmuls + sex = exp(SCALE*rstd1*logit) * c2f ----
        for st in sts:
            for lt in range(2):
                psSt = _pA(f"psS_{g}_{st}{lt}")
                psS = psSt[:LC, 0, :K]
                for dh in range(2):
                    nc.tensor.matmul(psS, lgT[:, st, dh, lt * LC : (lt + 1) * LC],
                                     c["m2tb"][:, dh, :], start=(dh == 0), stop=(dh == 1))
                nc.scalar.activation(out=sex[:LC, lt, st, :], in_=psS, func=AF.Exp,
                                     bias=0.0, scale=rs1s[:LC, lt, st, 0:1])
        # batched alpha chain on [:LC]
        c2b = _bins(_bins(c["c2fb"][:LC, :], 1, 2), 2, 2 * sgrp)
        for ssl in (gssl,):
            sexs = sex[:LC, :, ssl, :]
            ccas = cca[:LC, :, ssl, :]
            nc.vector.tensor_tensor(out=sexs, in0=sexs, in1=c2b, op=OP.mult)
            ssm = tmp.tile([LC, 2, 2 * sgrp, 1], F32, name=f"ssm_{g}", tag="ssm", bufs=2)
            nc.vector.reduce_sum(out=ssm, in_=sexs, axis=AX)
            src_ = tmp.tile([LC, 2, 2 * sgrp, 1], F32, name=f"src_{g}", tag="src", bufs=2)
            nc.vector.reciprocal(src_, ssm)
            nc.vector.tensor_tensor(out=ccas, in0=sexs, in1=_b0(src_[:, :, :, 0:1]),
                                    op=OP.mult)
            nc.vector.tensor_tensor(out=ccas, in0=ccas, in1=_b0a(wn[:, :, ssl]),
                                    op=OP.mult)
            sq = tmp.tile([LC, 2, 2 * sgrp, K], F32, name=f"sq_{g}", tag="sq", bufs=2)
            nc.vector.tensor_mul(sq, ccas, ccas)
            nc.vector.tensor_tensor(out=sq, in0=sq,
                                    in1=_b0(mv1[:LC, :, ssl, 1:2]), op=OP.mult)
            # (kept as two ops: stt scalar slot needs a single column)
            nc.scalar.activation(out=sq, in_=sq, func=AF.Ln, bias=c["epsc"][:LC, :],
                                 scale=1.0)
            nc.scalar.activation(out=sq, in_=sq, func=AF.Exp, bias=0.0, scale=-0.5)
            nc.vector.tensor_tensor(out=alf[:LC, :, ssl, :], in0=ccas, in1=sq,
                                    op=OP.mult)
        if upto <= 9:
            continue

        # ---- final: PE diag matmuls -> PSUM -> copy/stt -> SBUF -> DMA ----
        n_pe = K - n_hyb
        for b in seqs:
            stL, stG = 2 * b, 2 * b + 1
            for lt in range(2):
                osb = op.tile([LC, K, D], F32, name=f"osb_{b}_{lt}", tag="osb", bufs=4 if deep else 3)
                dgB = None
                if batch_diag and n_pe:
                    # one wide DVE op builds all (branch, k) diag stationaries
                    dgB = tmp.tile([LC + 1, 2, n_pe, LC], BF16,
                                   name=f"dgB_{b}_{lt}", tag="dgB", bufs=2)
                    nc.vector.tensor_tensor(
                        out=dgB,
                        in0=_bins(_bins(c["eyex1"], 1, 2), 2, n_pe),
                        in1=_b0a(alf[: LC + 1, lt, 2 * b : 2 * b + 2, 0:n_pe], LC),
                        op=OP.mult)
                def _emit_chunks():
                    for kc in range(0, n_pe, 2):
                        p10 = _pA(f"p10_{b}_{lt}{kc}")[:LC]
                        for ki in range(2):
                            k = kc + ki
                            if dgB is not None:
                                dgL, dgG = dgB[:, 0, k, :], dgB[:, 1, k, :]
                            else:
                                dgL = tmp.tile([LC + 1, LC], BF16, name=f"dgL_{b}_{lt}{k}",
                                               tag="dg", bufs=12 if deep else 6)
                                _diag(dgL, c["eyex1"], alf[: LC + 1, lt, stL, k : k + 1])
                                dgG = tmp.tile([LC + 1, LC], BF16, name=f"dgG_{b}_{lt}{k}",
                                               tag="dg", bufs=12 if deep else 6)
                                _diag(dgG, c["eyex1"], alf[: LC + 1, lt, stG, k : k + 1])
                            nc.tensor.matmul(p10[:, ki, :], dgL, lng[: LC + 1, lt, stL, :],
                                             start=True, stop=False)
                            nc.tensor.matmul(p10[:, ki, :], dgG, lng[: LC + 1, lt, stG, :],
                                             start=False, stop=True)
                        _copy10(osb[:, kc : kc + 2, :], p10)
                def _emit_hyb():
                    p1hts = [_pA(f"p1h_{b}_{lt}_{i}") for i in range((n_hyb + 1) // 2)]
                    for k in range(n_pe, K):
                        kk = k - n_pe
                        p1h = p1hts[kk // 2][:LC, kk % 2, :]
                        dgG = tmp.tile([LC + 1, LC], BF16, name=f"dgGh_{b}_{lt}{k}",
                                       tag="dg", bufs=12 if deep else 6)
                        _diag(dgG, c["eyex2"], alf[: LC + 1, lt, stG, k : k + 1])
                        nc.tensor.matmul(p1h, dgG, lng[: LC + 1, lt, stG, :],
                                         start=True, stop=True)
                        nc.vector.scalar_tensor_tensor(
                            out=osb[:, k, :], in0=lng[:LC, lt, stL, :],
                            scalar=alf[:LC, lt, stL, k : k + 1], in1=p1h,
                            op0=OP.mult, op1=OP.add)
                segs = ([_emit_hyb, _emit_chunks] if hyb_first
                        else [_emit_chunks, _emit_hyb])
                for fn in segs:
                    fn()
                if dma_out:
                    if dma_split == 2:
                        for kh in range(2):
                            ap = _dram_out_ap(out_t, b, lt)
                            ap = bass.AP(tensor=ap.tensor,
                                         offset=ap.offset + kh * 4 * L * D,
                                         ap=[list(ap.ap[0]), [L * D, 4],
                                             list(ap.ap[2])])
                            nc.sync.dma_start(out=ap, in_=osb[:, 4 * kh : 4 * kh + 4, :])
                    else:
                        nc.sync.dma_start(out=_dram_out_ap(out_t, b, lt), in_=osb)
                else:
                    nc.sync.dma_start(out=bass.AP(tensor=out_t, offset=0,
                                                  ap=[[D, 1], [1, 4]]),
                                      in_=osb[0:1, 0, 0:4])


def build_module(reps=1, upto=12, timing=False, n_hyb=2, diag_eng="dve", sgrp=2,
                 dma_out=True, diag_mix=("dve",), copy_mix=("act",), pair_t=False,
                 pbufs=(1, 2, 4), batch_diag=False, dma_split=1, deep=False,
                 hyb_first=True):
    nc = bacc.Bacc("TRN2", target_bir_lowering=False, debug=False,
                   num_devices=NCORES)
    big = "Internal" if timing else "ExternalInput"
    t = {}
    t["xL"] = nc.dram_tensor("xL", [BPC, L, D], F32, kind=big)
    t["xG"] = nc.dram_tensor("xG", [BPC, L, D], F32, kind=big)
    t["bf32"] = nc.dram_tensor("bf32", [128, CF32], F32, kind="ExternalInput")
    t["bb16"] = nc.dram_tensor("bb16", [128, CB16], BF16, kind="ExternalInput")
    t["wwt"] = nc.dram_tensor("wwt", [D, D], F32R, kind="ExternalInput")
    t["b5rep"] = nc.dram_tensor("b5rep", [5, 2, NS, D], BF16, kind="ExternalInput")
    t["qraw"] = nc.dram_tensor("qraw", [BPC, 2, D], F32, kind="ExternalInput")
    t["posrou"] = nc.dram_tensor("posrou", [BPC, D], F32, kind="ExternalInput")
    out_t = nc.dram_tensor("out", [BPC, K, L, D], F32,
                           kind="Internal" if timing else "ExternalOutput")
    sink_t = None
    if timing:
        sink_t = nc.dram_tensor("sink", [1, 4], F32, kind="ExternalOutput")

    with tile.TileContext(nc) as tc:
        with ExitStack() as ctx:
            cst = _emit_consts(nc, tc, ctx, t)
            blk = ctx.enter_context(tc.tile_pool(name="blk", bufs=1))
            tmp = ctx.enter_context(tc.tile_pool(name="tmp", bufs=2))
            pst = ctx.enter_context(tc.tile_pool(name="pst", bufs=pbufs[0], space="PSUM"))
            psw = ctx.enter_context(tc.tile_pool(name="psw", bufs=pbufs[1], space="PSUM"))
            psA = ctx.enter_context(tc.tile_pool(name="psA", bufs=pbufs[2], space="PSUM"))
            op = ctx.enter_context(tc.tile_pool(name="outp", bufs=1))
            pools = (blk, tmp, pst, psw, psA, op)
            if reps == 1:
                _emit_body(nc, tc, cst, pools, t, out_t, upto, n_hyb, diag_eng,
                           sgrp, dma_out, diag_mix, copy_mix, pair_t, batch_diag,
                           dma_split, deep, hyb_first)
            else:
                with tc.For_i(0, reps, 1):
                    _emit_body(nc, tc, cst, pools, t, out_t, upto, n_hyb,
                               diag_eng, sgrp, dma_out, diag_mix, copy_mix, pair_t,
                               batch_diag, dma_split, deep, hyb_first)
            if sink_t is not None:
                snk = tmp.tile([1, 4], F32, name="snk", tag="snk", bufs=1)
                nc.sync.dma_start(out=snk, in_=out_t[0, 0, 0:1, 0:4])
                nc.sync.dma_start(out=sink_t[:, :], in_=snk)
    nc.compile()
    return nc


def host_inputs(local_item_emb, global_item_emb, intentions, pos_fai, rou, W_w, W_b,
                g1, b1, g2, b2, g3, b3, g4, b4, g5, b5, seq_len):
    """Host-side param folding + per-core sharding. Returns in_maps list."""
    import ml_dtypes
    f = np.float32
    bf = ml_dtypes.bfloat16
    xL = np.ascontiguousarray(local_item_emb, f)
    xG = np.ascontiguousarray(global_item_emb, f)
    g1, b1, g2, b2 = (np.asarray(v, f) for v in (g1, b1, g2, b2))
    g3, b3, g4, b4 = (np.asarray(v, f) for v in (g3, b3, g4, b4))
    g5, b5 = np.asarray(g5, f), np.asarray(b5, f)
    intentions = np.asarray(intentions, f)
    mu = intentions.mean(-1, keepdims=True)
    var = ((intentions - mu) ** 2).mean(-1, keepdims=True)
    ln2 = (intentions - mu) / np.sqrt(var + EPS) * g2 + b2          # [K, D]
    assert np.abs(g5).min() > 1e-3, "g5 too small for m2 folding"
    m2 = np.ascontiguousarray((ln2 * (g1 / g5)[None, :]).T, f)      # [D, K]
    c2 = (ln2 @ b1.astype(np.float64)).astype(f).reshape(1, K)
    c2f = np.exp(SCALE * c2).astype(f)
    eye = np.eye(LC, dtype=f)
    eyex1 = np.concatenate([eye, np.ones((1, LC), f)], 0)
    eyex2 = np.concatenate([eye, 2.0 * np.ones((1, LC), f)], 0)
    b5rep = np.ascontiguousarray(np.broadcast_to(b5.reshape(1, 1, 1, D), (5, 2, NS, D))).astype(bf)
    sl = np.asarray(seq_len).astype(np.int64).reshape(B)
    idx = sl - 1
    posrou_all = (np.asarray(pos_fai, f)[idx] + np.asarray(rou, f)[None, :])  # [B, D]
    pos = np.asarray(pos_fai, f)
    named = {
        "pos": pos.reshape(2, LC, D).transpose(1, 0, 2),
        "g5bc": np.broadcast_to(g5.reshape(1, D), (LC, D)),
        "c2fb": np.broadcast_to(c2f, (LC, K)),
        "g3bc": np.broadcast_to(g3.reshape(1, D), (BPC, D)),
        "b3bc": np.broadcast_to(b3.reshape(1, D), (BPC, D)),
        "g4": g4.reshape(2, 128).T, "b4": b4.reshape(2, 128).T,
        "wb": np.asarray(W_b, f).reshape(2, 128).T,
        "eyef": eye,
        "m2tb": m2.reshape(2, 128, K).transpose(1, 0, 2),
        "eyeb": eye, "eyex1": eyex1, "eyex2": eyex2,
    }
    bf32_blob, bb16_blob = _pack_consts(named)
    shared = {
        "bf32": bf32_blob, "bb16": bb16_blob,
        "wwt": np.ascontiguousarray(np.asarray(W_w, f).T),
        "b5rep": b5rep,
    }
    in_maps = []
    for cix in range(NCORES):
        s = slice(cix * BPC, (cix + 1) * BPC)
        qraw = np.stack([xL[s][np.arange(BPC), idx[s]],
                         xG[s][np.arange(BPC), idx[s]]], axis=1)     # [BPC, 2, D]
        in_maps.append({"xL": xL[s], "xG": xG[s],
                        "qraw": np.ascontiguousarray(qraw, f),
                        "posrou": np.ascontiguousarray(posrou_all[s], f),
                        **shared})
    return in_maps


_module_cache = {}


def kernel(**inputs) -> np.ndarray:
    in_maps = host_inputs(**inputs)
    if 1 not in _module_cache:
        _module_cache[1] = build_module(reps=1)
    nc = _module_cache[1]
    r = run_bass_kernel_spmd(nc, in_maps, list(range(NCORES)))
    out = np.concatenate([r.results[cix]["out"] for cix in range(NCORES)], axis=0)
    return out.astype(np.float32)
